# revision 33
# baseline (speedup 1.0000x reference)
"""GAT attention head (gnn_message_passing) on 8 trn2 NeuronCores — v3.

Math (per batch b):
    F = seq @ W^T                        [N, 64]
    f1 = F @ a1^T + a1_b                 [N, 1]
    f2 = F @ a2^T + a2_b                 [N, 1]
    logits[i,j] = lrelu(f1[i] + f2[j]) + bias_mat[i,j]
    out = softmax_j(logits) @ F + bias   [N, 64]

Sharding: rows i are split across 8 cores (1024 rows each, both batches).
Each core redundantly computes the full F (cheap).

Two build variants, selected by a host-side all-zero test on bias_mat:

* zero_bias=False (general): streams the core's [2, 1024, 8192] shard
  of bias_mat, pre-transposed + cast to bf16 on the host; one fused DVE
  pass per chunk computes lrelu(f1+f2)+bias, ACT exponentiates, PE
  accumulates [F|1]^T @ E.  ACT(113us) and DMA(94us) bound.

* zero_bias=True + fact_k (fast path for the all-zero bias the problem
  ships): no bias stream at all.  Per 8-chunk j-group, fact_k chunks are
  "factorized": exp(lrelu(f1+f2)) = M*u_i*v_j + (1-M)*p_i*q_j with
  M = [f1+f2 >= 0], u=e^f1, v=e^f2, p=e^.01f1, q=e^.01f2 — so instead of
  exp on ACT they need only a stock 4x-mode is_ge (DVE) and two extra PE
  matmuls against precomputed [vF|qF] / [0|v|q] stationaries (PSUM rows
  0..127 of ACp and 64..66 of OT).  The per-i scalings u_i/p_i and the
  complement totals (Tq = sum_fact q*F) are applied in the epilogue
  after the transpose, where i is the partition axis.  The remaining
  (8-fact_k) chunks per group run the direct exp path, keeping ACT and
  PE balanced: fact_k=3 puts both at ~70us/pass (ACT exps 16x4452ns
  back-to-back in the cost-model timeline).

Per (b, j-group) steady-state (transposed layout: j partitions, i free):
    S[j,g*1024+i]: g<k   M = is_ge(f1, -f2)      (DVE stock 4x)
                   g>=k  lrelu(f1+f2)            (custom DVE 2x op)
    E = exp(S[:, k*1024:])                       (ACT, one pass)
    OT[0:65]  += [F|1]^T @ E                     (PE, direct chunks)
    ACp, OT[64:67] += [vF|qF]^T @ M, [0|v|q]^T @ M   (PE, fact chunks)
"""

import sys
import numpy as np
from contextlib import ExitStack

sys.path.insert(0, "/opt/trn_rl_repo")

import concourse.bass as bass
import concourse.tile as tile
from concourse import mybir, bacc, masks
from concourse.bass_utils import run_bass_kernel_spmd
import concourse.dve_ops as _D
from concourse.dve_spec import Spec as _Spec, Src0 as _Src0, Src1 as _Src1, \
    C0 as _C0, C1 as _C1, maxx as _maxx, lower as _lower, Zero as _Zero
from concourse.dve_uop import DveOpSpec as _DveOpSpec


def _make_2x_uop(u1x):
    """Hand-built 2X_1PORT program for the fused score op.

    Processes a packed bf16 pair per cycle: lo chain on ALU stages 0-3
    (the same schedule lower() emits for the 1x program), hi chain on
    stages 4-7 reading the SRC_*_HI crossbar lanes.  The lo result is
    parked on delay lane 0 at stage 4 and rides to the output flops, so
    both halves exit on the same wavefront: WR0_LO <- DELAY_0 (lo),
    WR0_HI <- ALU_OUT (hi).  Constants (per-partition f2 scalar, slope)
    are shared by both chains via delay lanes 1-2.
    """
    import copy as _copy
    from concourse.dve_uop import (UopConfig as _UC, UopDpConfig as _DP,
                                   AluInp as _AI, DelayInp as _DI,
                                   InpSel as _IS, OutSel as _OS,
                                   OutPath as _OP)
    AluOp = type(u1x.datapath_config[0].op)
    PD, PA = _DI.PREV_DELAY, _DI.PREV_ALU_OUT
    u = _copy.deepcopy(u1x)
    # crossbar: lanes 0-5 feed delay chains 0-5
    u.inp = [_IS.ZERO] * len(u.inp)
    u.inp_enable = [0] * len(u.inp_enable)
    for lane, sel in ((0, _IS.SRC_1), (1, _IS.CONST_0), (2, _IS.CONST_1),
                      (3, _IS.SRC_0), (4, _IS.SRC_1_HI), (5, _IS.SRC_0_HI)):
        u.inp[lane + 1] = sel
        u.inp_enable[lane + 1] = 1

    den = [1, 1, 1, 1, 1, 1, 0]

    def dp(op, a, b, l0=PD, l4=PD):
        delays = [l0, PD, PD, PD, l4, PD, PD]
        return _DP(op=op, alu_src0=a, alu_src1=b, delay=delays,
                   alu_out_enable=1, swap_enable=0, alu_out_a_enable=0,
                   alu_out_b_enable=0, delay_enable=list(den),
                   idx0_sel=0, idx1_sel=0)

    u.datapath_config = [
        # lo: s = Src1 + C0 ; m = s*C1 ; x = max(s, m) ; r = x + Src0
        dp(AluOp.ADD, _AI.PREV_DELAY_0, _AI.PREV_DELAY_1),
        dp(AluOp.MULTIPLY, _AI.PREV_ALU_OUT, _AI.PREV_DELAY_2, l0=PA),
        dp(AluOp.MAX, _AI.PREV_DELAY_0, _AI.PREV_ALU_OUT),
        dp(AluOp.ADD, _AI.PREV_ALU_OUT, _AI.PREV_DELAY_3),
        # hi: same ops on SRC_*_HI; park r_lo on delay lane 0
        dp(AluOp.ADD, _AI.PREV_DELAY_4, _AI.PREV_DELAY_1, l0=PA),
        dp(AluOp.MULTIPLY, _AI.PREV_ALU_OUT, _AI.PREV_DELAY_2, l4=PA),
        dp(AluOp.MAX, _AI.PREV_DELAY_4, _AI.PREV_ALU_OUT),
        dp(AluOp.ADD, _AI.PREV_ALU_OUT, _AI.PREV_DELAY_5),
    ]
    u.out = {o: _OS.ALU_OUT for o in _OP}
    u.out_enable = {o: 0 for o in _OP}
    u.out[_OP.WR0_LO] = _OS.DELAY_0
    u.out_enable[_OP.WR0_LO] = 1
    u.out[_OP.WR0_HI] = _OS.ALU_OUT
    u.out_enable[_OP.WR0_HI] = 1
    return u


def _register_op(name, body, reference, make2x, rd1_en=True):
    for op in _D.OPS:
        if op.name == name:
            return op
    spec = _Spec(body=body, reference=reference)
    opcode = _D._CUSTOM_DVE_ROW_BASE + len(_D.OPS)
    shas = {}
    specs = {}
    for ver in ("v3", "v4"):
        uops = _lower(spec, ver=ver)
        u2x = make2x(uops[0])
        u2x.validate(ver)
        dspec = _DveOpSpec(name=name, opcode=opcode, uops=uops,
                           uops_2x=[u2x], rd1_en=rd1_en)
        shas[ver] = dspec.sha(ver)
        specs[ver] = dspec
    op = _D.DveOp(name, spec, subdim=False, uops_sha=shas)
    _D.OPS.append(op)
    _D._SUB_OPCODE_FOR_NAME[name] = opcode
    _D.CUSTOM_DVE_SPECS[name] = spec
    # DveOp.compile() re-lowers from the Spec (1x only); pre-seed its
    # memoisation cache so table-gen picks up the 2X_1PORT variant.
    for ver, dspec in specs.items():
        _D._COMPILE_CACHE[(name, ver)] = dspec
    return op


def _register_fused_op():
    """out = lrelu(in1 + s0) + in0, slope s1 -- one DVE pass for the whole
    score stage (outer-sum + leaky-relu + bias add).  Registers a
    hand-built 2X_1PORT uop variant so packed bf16 operands run at
    2 elem/cycle/lane (the stock lower() only emits the 1x program)."""
    s = _Src1 + _C0
    return _register_op(
        "GAT_SCORE_FUSED2X_ANT", _maxx(s, s * _C1) + _Src0,
        lambda in0, in1, s0, s1, imm2:
            np.maximum(in1 + s0, (in1 + s0) * s1) + in0,
        _make_2x_uop)


def _make_2x_uop_nobias(u1x):
    """2X_1PORT program for out = lrelu(in1 + s0): the fused op's 2x
    program with the final +Src0 stages (3 lo, 7 hi) turned into identity
    max(x, x).  Crossbar layout unchanged (rd0 still streams but is
    ignored), so the op stays formally two-source and uses the proven
    TwoSrc/2X_1PORT table slot."""
    from concourse.dve_uop import AluInp as _AI
    u = _make_2x_uop(u1x)
    AluOp = type(u.datapath_config[0].op)
    for st in (3, 7):
        dp = u.datapath_config[st]
        dp.op = AluOp.MAX
        dp.alu_src0 = _AI.PREV_ALU_OUT
        dp.alu_src1 = _AI.PREV_ALU_OUT
    return u


def _register_nobias_op():
    """out = lrelu(in1 + s0), slope s1 -- the zero-bias score stage.  in0
    is a dummy operand (streamed, unused) so the op keeps the TwoSrc
    perf-mode path."""
    s = _Src1 + _C0
    return _register_op(
        "GAT_SCORE_NOBIAS2X_ANT", _maxx(s, s * _C1) + _Src0 * _Zero,
        lambda in0, in1, s0, s1, imm2:
            np.maximum(in1 + s0, (in1 + s0) * s1),
        _make_2x_uop_nobias)


FUSED_OP = _register_fused_op()
NOBIAS_OP = _register_nobias_op()

def _set_perf_max(inst, v):
    """Set perf_max on the emitted InstCustomDveAnt (wrapper-agnostic)."""
    for obj in (inst, getattr(inst, "ins", None),
                getattr(inst, "instruction", None)):
        if obj is not None and hasattr(obj, "perf_max"):
            obj.perf_max = v
            return
    raise AttributeError(f"perf_max not found on {type(inst)}")



FP = mybir.dt.float32
BF = mybir.dt.bfloat16
NPBF = mybir.dt.np(mybir.dt.bfloat16)
B, N, IN, OUT = 2, 8192, 128, 64
NCORES = 8
LOCAL = N // NCORES          # 1024 rows per core per batch
NCH = N // 128               # 64 j-chunks per batch
JG = 8                       # j-chunks per score-tile group
NGRP = NCH // JG             # groups per batch
NEG = 0.01                   # leaky relu slope
MMW = 1024                   # matmul free width (PSUM write span)

_CACHED_NC = {}


def build_nc(p2_repeat=1, sbufs=5, do_dma=True, do_elem=True, do_exp=True,
             do_mm=True, zero_bias=False, fact_k=0):
    assert fact_k == 0 or zero_bias, "factorized chunks require zero bias"
    nc = bacc.Bacc("TRN2", target_bir_lowering=False, debug=False,
                   num_devices=NCORES)

    # per-core inputs (host prepares layouts; all pure layout transforms)
    # biasT[b, jg, p, g*LOCAL + i] = bf16(bias_mat[b, row, jg*1024 + g*128 + p])
    if not zero_bias:
        biasT = nc.dram_tensor("biasT", [B, NGRP, 128, JG * LOCAL], BF,
                               kind="ExternalInput").ap()
    seqT = nc.dram_tensor("seqT", [B, IN, N], BF, kind="ExternalInput").ap()
    seqlT = nc.dram_tensor("seqlT", [B, IN, LOCAL], BF, kind="ExternalInput").ap()
    Wn = nc.dram_tensor("Wn", [OUT, IN], FP, kind="ExternalInput").ap()
    WTb = nc.dram_tensor("WTb", [IN, OUT], BF, kind="ExternalInput").ap()
    a1T = nc.dram_tensor("a1T", [OUT, 1], FP, kind="ExternalInput").ap()
    a2T = nc.dram_tensor("a2T", [OUT, 1], FP, kind="ExternalInput").ap()
    a1b = nc.dram_tensor("a1b", [1, 1], FP, kind="ExternalInput").ap()
    a2b = nc.dram_tensor("a2b", [1, 1], FP, kind="ExternalInput").ap()
    brow = nc.dram_tensor("brow", [1, OUT], FP, kind="ExternalInput").ap()
    out = nc.dram_tensor("out", [B, LOCAL, OUT], FP, kind="ExternalOutput").ap()

    with tile.TileContext(nc) as tc, ExitStack() as ctx:
        # ---- persistent pools -------------------------------------------
        const_pool = ctx.enter_context(tc.tile_pool(name="const", bufs=1))
        feat_pool = ctx.enter_context(tc.tile_pool(name="feat", bufs=1))
        spool = ctx.enter_context(tc.tile_pool(name="scores", bufs=sbufs))
        tpool = ctx.enter_context(tc.tile_pool(name="tmp", bufs=3))
        opool = ctx.enter_context(tc.tile_pool(name="outs", bufs=2))
        setup_ctx = ctx.enter_context(ExitStack())
        ps_one = setup_ctx.enter_context(
            tc.tile_pool(name="ps_one", bufs=1, space="PSUM"))
        ps_small = setup_ctx.enter_context(
            tc.tile_pool(name="ps_small", bufs=2, space="PSUM"))

        # ---- P0: constants ----------------------------------------------
        ident = const_pool.tile([128, 128], FP)
        masks.make_identity(nc, ident[:])
        ones_row = const_pool.tile([1, 128], FP)
        nc.vector.memset(ones_row[:], 1.0)
        ones_row_bf = const_pool.tile([1, 128], BF)
        nc.vector.memset(ones_row_bf[:], 1.0)
        ones_col_bf = const_pool.tile([128, 1], BF)
        nc.vector.memset(ones_col_bf[:], 1.0)

        Wsb = const_pool.tile([OUT, IN], FP)
        nc.sync.dma_start(Wsb[:], Wn[:])
        rhs66 = const_pool.tile([IN, 66], BF)
        nc.sync.dma_start(rhs66[:, 0:64], WTb[:])
        a1sb = const_pool.tile([OUT, 1], FP)
        nc.sync.dma_start(a1sb[:], a1T[:])
        a2sb = const_pool.tile([OUT, 1], FP)
        nc.sync.dma_start(a2sb[:], a2T[:])
        a1bsb = const_pool.tile([1, 1], FP)
        nc.sync.dma_start(a1bsb[:], a1b[:])
        a2bsb = const_pool.tile([1, 1], FP)
        nc.sync.dma_start(a2bsb[:], a2b[:])
        brsb = const_pool.tile([1, OUT], FP)
        nc.sync.dma_start(brsb[:], brow[:])

        # w1 = W^T @ a1 (column IN-vector), w2 likewise -> rhs66 cols 64, 65
        wps = ps_one.tile([IN, 2], FP)
        nc.tensor.matmul(wps[:, 0:1], Wsb[:], a1sb[:], start=True, stop=True)
        nc.tensor.matmul(wps[:, 1:2], Wsb[:], a2sb[:], start=True, stop=True)
        nc.vector.tensor_copy(rhs66[:, 64:66], wps[:])

        # broadcasts across partitions (matmul with ones stationary)
        bps = ps_one.tile([128, OUT + 2], FP)
        nc.tensor.matmul(bps[:, 0:1], ones_row[:], a1bsb[:], start=True, stop=True)
        nc.tensor.matmul(bps[:, 1:2], ones_row[:], a2bsb[:], start=True, stop=True)
        nc.tensor.matmul(bps[:, 2:2 + OUT], ones_row[:], brsb[:], start=True, stop=True)
        a1b_bc = const_pool.tile([128, 1], FP)
        a2b_bc = const_pool.tile([128, 1], FP)
        bias_bc = const_pool.tile([128, OUT], FP)
        nc.vector.tensor_copy(a1b_bc[:], bps[:, 0:1])
        nc.vector.tensor_copy(a2b_bc[:], bps[:, 1:2])
        nc.vector.tensor_copy(bias_bc[:], bps[:, 2:2 + OUT])

        # ---- P1: features -----------------------------------------------
        # Vp: per j-chunk [128, 65] = [F chunk | ones]; both batches.  bf16.
        Vp = feat_pool.tile([128, B * NCH * 65], BF)
        Vp3 = Vp[:].rearrange("p (n e) -> p n e", e=65)
        nc.vector.memset(Vp3[:, :, 64:65], 1.0)
        f2T = feat_pool.tile([128, B * NCH], FP)
        F1B = feat_pool.tile([128, B * LOCAL], BF)

        # Process 4 j-chunks per PSUM tile; batch the PSUM->SBUF copies.
        for b in range(B):
            for jq in range(NCH // 4):
                nt0 = b * NCH + jq * 4
                sqt = tpool.tile([128, 512], BF, tag="sqt")
                nc.sync.dma_start(sqt[:], seqT[b][:, jq * 512:(jq + 1) * 512])
                fc4 = ps_small.tile([128, 4 * 66], FP, tag="fc")
                fc4v = fc4[:].rearrange("p (c e) -> p c e", c=4)
                for c in range(4):
                    nc.tensor.matmul(fc4v[:, c, :], sqt[:, c * 128:(c + 1) * 128],
                                     rhs66[:], start=True, stop=True)
                nc.vector.tensor_copy(Vp3[:, nt0:nt0 + 4, 0:64], fc4v[:, :, 0:64])
                f2Tv = f2T[:].rearrange("p (n e) -> p n e", e=1)
                nc.vector.tensor_scalar(f2Tv[:, nt0:nt0 + 4, :], fc4v[:, :, 65:66],
                                        a2b_bc[:], None, mybir.AluOpType.add)

        if fact_k > 0:
            # factorized-chunk machinery: exp(lrelu(f1+f2)) on a fact chunk
            # = M*u_i*v_j + (1-M)*p_i*q_j with M = [f1_i + f2_j >= 0].
            negf2T = feat_pool.tile([128, B * NCH], FP)
            nc.vector.tensor_scalar(negf2T[:], f2T[:], -1.0, None,
                                    mybir.AluOpType.mult)
            vT = feat_pool.tile([128, B * NCH], FP)
            qT = feat_pool.tile([128, B * NCH], FP)
            nc.scalar.activation(vT[:], f2T[:], mybir.ActivationFunctionType.Exp)
            nc.scalar.activation(qT[:], f2T[:], mybir.ActivationFunctionType.Exp,
                                 scale=NEG)
            # VQp: per chunk [128, 128] = [v*F | q*F]; dvq: [v | q]; dq0: [0 | q]
            VQp = feat_pool.tile([128, B * NCH * 128], BF)
            VQp3 = VQp[:].rearrange("p (n e) -> p n e", e=128)
            dvq = feat_pool.tile([128, B * NCH * 3], BF)
            dvq3 = dvq[:].rearrange("p (n e) -> p n e", e=3)
            dq0 = feat_pool.tile([128, B * NCH * 3], BF)
            dq03 = dq0[:].rearrange("p (n e) -> p n e", e=3)
            nc.vector.memset(dvq3[:, :, 0:1], 0.0)
            nc.vector.memset(dq03[:, :, 0:2], 0.0)
            nc.vector.tensor_copy(dvq3[:, :, 1:2],
                                  vT[:].rearrange("p (n e) -> p n e", e=1))
            nc.vector.tensor_copy(dvq3[:, :, 2:3],
                                  qT[:].rearrange("p (n e) -> p n e", e=1))
            nc.vector.tensor_copy(dq03[:, :, 2:3],
                                  qT[:].rearrange("p (n e) -> p n e", e=1))
            for nt in range(B * NCH):
                nc.vector.tensor_scalar(VQp3[:, nt, 0:64], Vp3[:, nt, 0:64],
                                        vT[:, nt:nt + 1], None,
                                        mybir.AluOpType.mult)
                nc.vector.tensor_scalar(VQp3[:, nt, 64:128], Vp3[:, nt, 0:64],
                                        qT[:, nt:nt + 1], None,
                                        mybir.AluOpType.mult)
            # Tq totals over the fact chunks of each batch:
            #   TqA_col[p in 64:128] = sum_f q_j F_j,(p-64);  TqO_col[97] = sum_f q_j
            TqA_cols, TqO_cols = [], []
            fact_nts = [jg * JG + g for jg in range(NGRP)
                        for g in range(fact_k)]
            for b in range(B):
                tqa = ps_small.tile([128, 1], FP, tag="fc")
                tqo = ps_small.tile([128, 1], FP, tag="f1bp")
                for i, fnt in enumerate(fact_nts):
                    nt = b * NCH + fnt
                    st, sp = (i == 0), (i == len(fact_nts) - 1)
                    nc.tensor.matmul(tqa[64:128, :], VQp3[:, nt, 64:128],
                                     ones_col_bf[:], start=st, stop=sp)
                    nc.tensor.matmul(tqo[64:67, :], dq03[:, nt, :],
                                     ones_col_bf[:], start=st, stop=sp,
                                     skip_group_check=True)
                tqa_sb = feat_pool.tile([128, 1], FP, tag=f"tqa_sb{b}")
                tqo_sb = feat_pool.tile([128, 1], FP, tag=f"tqo_sb{b}")
                nc.vector.memset(tqa_sb[:], 0.0)
                nc.vector.memset(tqo_sb[:], 0.0)
                nc.vector.tensor_copy(tqa_sb[64:128, :], tqa[64:128, :])
                nc.vector.tensor_copy(tqo_sb[64:67, :], tqo[64:67, :])
                TqA_cols.append(tqa_sb)
                TqO_cols.append(tqo_sb)
            u_t = feat_pool.tile([128, B * (LOCAL // 128)], FP)
            p_t = feat_pool.tile([128, B * (LOCAL // 128)], FP)

        for b in range(B):
            for il in range(LOCAL // 128):
                slt = tpool.tile([128, 128], BF, tag="sqt")
                nc.sync.dma_start(slt[:], seqlT[b][:, il * 128:(il + 1) * 128])
                flc = ps_small.tile([128, 66], FP, tag="fc")
                nc.tensor.matmul(flc[:], slt[:], rhs66[:], start=True, stop=True)
                f1c = tpool.tile([128, 1], FP, tag="f1c")
                nc.vector.tensor_scalar(f1c[:], flc[:, 64:65], a1b_bc[:], None,
                                        mybir.AluOpType.add)
                if fact_k > 0:
                    c = b * (LOCAL // 128) + il
                    nc.scalar.activation(u_t[:, c:c + 1], f1c[:],
                                         mybir.ActivationFunctionType.Exp)
                    nc.scalar.activation(p_t[:, c:c + 1], f1c[:],
                                         mybir.ActivationFunctionType.Exp,
                                         scale=NEG)
                f1ct = ps_small.tile([1, 128], FP, tag="f1ct")
                nc.tensor.transpose(f1ct[:], f1c[:], ident[:])
                f1cs = tpool.tile([1, 128], FP, tag="f1cs")
                nc.vector.tensor_copy(f1cs[:], f1ct[:])
                f1bp = ps_small.tile([128, 128], FP, tag="f1bp")
                nc.tensor.matmul(f1bp[:], ones_row[:], f1cs[:], start=True, stop=True)
                nc.vector.tensor_copy(
                    F1B[:, b * LOCAL + il * 128: b * LOCAL + (il + 1) * 128],
                    f1bp[:])

        # ---- P2: main loop ----------------------------------------------
        setup_ctx.close()  # release P0/P1 PSUM banks
        ndir = JG - fact_k  # direct chunks per group (tail of each group)
        ps_loop = ExitStack()
        ps_ot = ps_loop.enter_context(
            tc.tile_pool(name="ps_ot", bufs=2, space="PSUM"))
        if fact_k > 0:
            ps_ac = ps_loop.enter_context(
                tc.tile_pool(name="ps_ac", bufs=2, space="PSUM"))
            ps_tr0 = None
        else:
            ps_tr0 = ctx.enter_context(
                tc.tile_pool(name="ps_tr", bufs=2, space="PSUM"))

        loop_ctx = ExitStack()
        if p2_repeat > 1:
            loop_ctx.enter_context(tc.For_i(0, p2_repeat, 1))

        OTps = {}
        ACps = {}

        def epilogue(b, ps_tr):
            # divide by rowsum, add bias, store
            OTp = OTps[b]
            ot_sb = opool.tile([65, LOCAL], FP, tag="ot_sb")
            nc.vector.tensor_copy(ot_sb[:], OTp[:])
            for r in range(LOCAL // 128):
                tp = ps_tr.tile([128, 65], FP, tag="tp")
                nc.tensor.transpose(tp[:], ot_sb[:, r * 128:(r + 1) * 128],
                                    ident[:65, :65])
                rcp = tpool.tile([128, 1], FP, tag="rcp")
                nc.vector.reciprocal(rcp[:], tp[:, 64:65])
                osb = opool.tile([128, OUT], FP, tag="osb")
                nc.vector.scalar_tensor_tensor(osb[:], tp[:, 0:64], rcp[:],
                                               bias_bc[:],
                                               mybir.AluOpType.mult,
                                               mybir.AluOpType.add)
                nc.sync.dma_start(out[b, r * 128:(r + 1) * 128, :], osb[:])

        def epilogue_sb(b):
            # stage psum -> SBUF with the Tq subtraction folded in (needs
            # the accumulation pools still open)
            ot_sb = opool.tile([67, LOCAL], FP, tag=f"ot_sb{b}")
            nc.vector.tensor_scalar(ot_sb[:], OTps[b][:], TqO_cols[b][0:67, :],
                                    None, mybir.AluOpType.subtract)
            ac_sb = opool.tile([128, LOCAL], FP, tag=f"ac_sb{b}")
            nc.vector.tensor_scalar(ac_sb[:], ACps[b][:], TqA_cols[b][:],
                                    None, mybir.AluOpType.subtract)
            return ot_sb, ac_sb

        def epilogue_hyb(b, ot_sb, ac_sb, ps_tr):
            # out = num/den + bias with
            #   -num = p*(C-TqF) - (u*A + OT[:,:64]);  -den likewise via
            #   den cols (OT 64 | Av 96 | Cq-Tqden 97); signs cancel in /.
            nb = LOCAL // 128
            for r in range(nb):
                sl = slice(r * 128, (r + 1) * 128)
                u_col = u_t[:, b * nb + r: b * nb + r + 1]
                p_col = p_t[:, b * nb + r: b * nb + r + 1]
                tpO = ps_tr.tile([128, 67], FP, tag="tpO")
                nc.tensor.transpose(tpO[:], ot_sb[:, sl], ident[:67, :67])
                tpAC = ps_tr.tile([128, 128], FP, tag="tpAC")
                nc.tensor.transpose(tpAC[:], ac_sb[:, sl], ident[:])
                tpOs = tpool.tile([128, 67], FP, tag="tpOs")
                nc.vector.tensor_copy(tpOs[:], tpO[:])
                s1 = tpool.tile([128, 65], FP, tag="s1")
                nc.vector.scalar_tensor_tensor(
                    s1[:, 0:64], tpAC[:, 0:64], u_col, tpOs[:, 0:64],
                    mybir.AluOpType.mult, mybir.AluOpType.add)
                nc.vector.scalar_tensor_tensor(
                    s1[:, 64:65], tpOs[:, 65:66], u_col, tpOs[:, 64:65],
                    mybir.AluOpType.mult, mybir.AluOpType.add)
                s2 = tpool.tile([128, 65], FP, tag="s2")
                nc.vector.scalar_tensor_tensor(
                    s2[:, 0:64], tpAC[:, 64:128], p_col, s1[:, 0:64],
                    mybir.AluOpType.mult, mybir.AluOpType.subtract)
                nc.vector.scalar_tensor_tensor(
                    s2[:, 64:65], tpOs[:, 66:67], p_col, s1[:, 64:65],
                    mybir.AluOpType.mult, mybir.AluOpType.subtract)
                rcp = tpool.tile([128, 1], FP, tag="rcp")
                nc.vector.reciprocal(rcp[:], s2[:, 64:65])
                osb = opool.tile([128, OUT], FP, tag="osb")
                nc.vector.scalar_tensor_tensor(osb[:], s2[:, 0:64], rcp[:],
                                               bias_bc[:],
                                               mybir.AluOpType.mult,
                                               mybir.AluOpType.add)
                nc.sync.dma_start(out[b, sl, :], osb[:])

        for b in range(B):
            OTp = ps_ot.tile([67 if fact_k else 65, LOCAL], FP)
            OTps[b] = OTp
            if fact_k > 0:
                ACp = ps_ac.tile([128, LOCAL], FP)
                ACps[b] = ACp
            F1Bb = F1B[:, b * LOCAL:(b + 1) * LOCAL]
            for jg in range(NGRP):
                S = spool.tile([128, JG * LOCAL], BF, tag="S")
                S3 = S[:].rearrange("p (g i) -> p g i", g=JG)
                if do_dma and not zero_bias:
                    nc.sync.dma_start(S[:], biasT[b, jg])
                elif not zero_bias:
                    nc.vector.memset(S[:, 0:1], 0.0)
                if do_elem:
                    # scores first so ACT's deps resolve before the masks
                    order = ([g for g in range(JG) if g >= fact_k] +
                             [g for g in range(fact_k)])
                    for g in order:
                        nt = b * NCH + jg * JG + g
                        if zero_bias and g < fact_k:
                            # factorized chunk: mask M = [f1_i >= -f2_j]
                            nc.vector.tensor_scalar(
                                S3[:, g], F1Bb, negf2T[:, nt:nt + 1], None,
                                mybir.AluOpType.is_ge)
                            continue
                        if zero_bias:
                            inst = nc.vector._custom_dve(
                                NOBIAS_OP, out=S3[:, g], in0=F1Bb,
                                in1=F1Bb, s0=f2T[:, nt:nt + 1], s1=NEG)
                        else:
                            inst = nc.vector._custom_dve(
                                FUSED_OP, out=S3[:, g], in0=S3[:, g],
                                in1=F1Bb, s0=f2T[:, nt:nt + 1], s1=NEG)
                        # expose the 2X_1PORT table slot (byte-36[7:6]);
                        # _custom_dve hardwires perf_max=0 (1x only)
                        _set_perf_max(inst, 1)
                if do_exp:
                    nc.scalar.activation(S[:, fact_k * LOCAL:],
                                         S[:, fact_k * LOCAL:],
                                         mybir.ActivationFunctionType.Exp)
                if do_mm:
                    for g in range(JG):
                        jc = jg * JG + g
                        nt = b * NCH + jc
                        if g >= fact_k:
                            lhsT = Vp3[:, nt, :]
                            st = (jg == 0 and g == fact_k)
                            sp = (jg == NGRP - 1 and g == JG - 1)
                            for h in range(LOCAL // MMW):
                                nc.tensor.matmul(
                                    OTp[0:65, h * MMW:(h + 1) * MMW], lhsT,
                                    S3[:, g, h * MMW:(h + 1) * MMW],
                                    start=st, stop=sp,
                                    skip_group_check=(fact_k > 0))
                        else:
                            st = (jg == 0 and g == 0)
                            sp = (jg == NGRP - 1 and g == fact_k - 1)
                            # both halves per stationary, so each lhsT
                            # loads once per chunk instead of twice
                            for h in range(LOCAL // MMW):
                                nc.tensor.matmul(
                                    ACps[b][:, h * MMW:(h + 1) * MMW],
                                    VQp3[:, nt, :],
                                    S3[:, g, h * MMW:(h + 1) * MMW],
                                    start=st, stop=sp)
                            for h in range(LOCAL // MMW):
                                nc.tensor.matmul(
                                    OTp[64:67, h * MMW:(h + 1) * MMW],
                                    dvq3[:, nt, :],
                                    S3[:, g, h * MMW:(h + 1) * MMW],
                                    start=st, stop=sp,
                                    skip_group_check=True)

            if p2_repeat == 1 and do_mm and fact_k == 0:
                epilogue(b, ps_tr0)

        loop_ctx.close()
        if do_mm and fact_k > 0:
            sb_pairs = [epilogue_sb(b) for b in range(B)]
            ps_loop.close()
            ps_tr = ctx.enter_context(
                tc.tile_pool(name="ps_tr2", bufs=3, space="PSUM"))
            for b in range(B):
                epilogue_hyb(b, *sb_pairs[b], ps_tr)
        elif p2_repeat != 1 and do_mm:
            for b in range(B):
                epilogue(b, ps_tr0)
        if not do_mm:
            # keep outputs written so the NEFF has valid outs
            for b in range(B):
                for r in range(LOCAL // 128):
                    osb = opool.tile([128, OUT], FP, tag="osb")
                    nc.vector.memset(osb[:], 0.0)
                    nc.sync.dma_start(out[b, r * 128:(r + 1) * 128, :], osb[:])

    nc.compile()
    return nc


FACT_K = 3  # factorized chunks per 8-chunk group on the zero-bias path


def get_nc(zero_bias=False):
    if zero_bias not in _CACHED_NC:
        _CACHED_NC[zero_bias] = build_nc(
            zero_bias=zero_bias, fact_k=FACT_K if zero_bias else 0)
    return _CACHED_NC[zero_bias]


def make_in_maps(inputs, zero_bias=False):
    seq = np.asarray(inputs["seq"], dtype=np.float32)
    bias_mat = np.asarray(inputs["bias_mat"], dtype=np.float32)
    W_w = np.ascontiguousarray(np.asarray(inputs["W_w"], dtype=np.float32))
    a1_w = np.asarray(inputs["a1_w"], dtype=np.float32)
    a1_b = np.asarray(inputs["a1_b"], dtype=np.float32)
    a2_w = np.asarray(inputs["a2_w"], dtype=np.float32)
    a2_b = np.asarray(inputs["a2_b"], dtype=np.float32)
    bias = np.asarray(inputs["bias"], dtype=np.float32)

    seqT = np.ascontiguousarray(seq.transpose(0, 2, 1).astype(NPBF))
    WTb = np.ascontiguousarray(W_w.T.astype(NPBF))
    a1T = np.ascontiguousarray(a1_w.reshape(OUT, 1))
    a2T = np.ascontiguousarray(a2_w.reshape(OUT, 1))
    a1b = a1_b.reshape(1, 1).astype(np.float32)
    a2b = a2_b.reshape(1, 1).astype(np.float32)
    brow = bias.reshape(1, OUT).astype(np.float32)

    in_maps = []
    for c in range(NCORES):
        sl = slice(c * LOCAL, (c + 1) * LOCAL)
        seqlT_c = np.ascontiguousarray(
            seq[:, sl, :].transpose(0, 2, 1).astype(NPBF))
        im = {
            "seqT": seqT, "seqlT": seqlT_c,
            "Wn": W_w, "WTb": WTb, "a1T": a1T, "a2T": a2T,
            "a1b": a1b, "a2b": a2b, "brow": brow,
        }
        if not zero_bias:
            shard = bias_mat[:, sl, :].reshape(B, LOCAL, NGRP, JG, 128)
            im["biasT"] = np.ascontiguousarray(
                shard.transpose(0, 2, 4, 3, 1).astype(NPBF)
            ).reshape(B, NGRP, 128, JG * LOCAL)
        in_maps.append(im)
    return in_maps


def kernel(**inputs) -> np.ndarray:
    zb = not np.any(np.asarray(inputs["bias_mat"]))
    nc = get_nc(zero_bias=zb)
    in_maps = make_in_maps(inputs, zero_bias=zb)
    res = run_bass_kernel_spmd(nc, in_maps, list(range(NCORES)))
    full = np.empty((B, N, OUT), dtype=np.float32)
    for c in range(NCORES):
        full[:, c * LOCAL:(c + 1) * LOCAL, :] = res.results[c]["out"]
    return full


if __name__ == "__main__":
    rng = np.random.default_rng(0)
    ins = {
        "seq": rng.standard_normal((B, N, IN), dtype=np.float32),
        "bias_mat": np.zeros((B, N, N), dtype=np.float32),
        "W_w": rng.standard_normal((OUT, IN), dtype=np.float32) * 0.05,
        "a1_w": rng.standard_normal((1, OUT), dtype=np.float32) * 0.05,
        "a1_b": rng.standard_normal((1,), dtype=np.float32) * 0.05,
        "a2_w": rng.standard_normal((1, OUT), dtype=np.float32) * 0.05,
        "a2_b": rng.standard_normal((1,), dtype=np.float32) * 0.05,
        "bias": np.zeros((OUT,), dtype=np.float32),
    }
    out = kernel(**ins)
    print("out", out.shape, out.dtype, float(np.abs(out).mean()))

